# revision 1
# baseline (speedup 1.0000x reference)
"""Trainium2 Bass kernel: dense transformer attention block (QKV proj + RoPE +
GQA causal attention + output proj), tensor-parallel over 8 NeuronCores.

Sharding: heads split across cores (4 Q heads + 1 KV head per core). Each core
computes its QKV shard for all tokens, runs attention for its heads, the
head-sharded attention outputs are AllGathered in 256-token chunks (overlapped
with compute), and each core computes a 512-column slice of the output
projection.

v3 layout: attention is kt-major and head-packed — for each 128-key tile, the
4 local heads' score matmuls share one stationary operand, EXP batches 2 heads
per ACT instruction, the softmax-denominator accumulation is one packed DVE add
per key tile, and the partition-reduction (ones-matmul broadcast) runs once per
(batch, 256-query chunk) instead of once per key tile. Causal masking uses
0/1 mask multiplies on the (otherwise idle) GpSimd engine. Emission is
software-pipelined: scores for key-tile kt+1 are emitted before PV of kt so
the in-order PE stream hides the EXP latency.
"""

from contextlib import ExitStack

import numpy as np
import ml_dtypes

import concourse.bass as bass
from concourse import bacc
import concourse.tile as tile
import concourse.mybir as mybir
from concourse.bass_utils import run_bass_kernel_spmd

F32 = mybir.dt.float32
F32R = mybir.dt.float32r
BF16 = mybir.dt.bfloat16
EXP = mybir.ActivationFunctionType.Exp

N_CORES = 8
N_HEADS = 32
N_KV_HEADS = 8
D = 128          # head dim
HID = 4096
B = 2
S = 2048
T = B * S        # 4096 tokens
ROPE_BASE = 10000.0

HL = N_HEADS // N_CORES          # 4 local Q heads per core
QKV_ROWS = (HL + 2) * D          # 768: 4 Q heads + 1 K head + 1 V head
JC = HID // N_CORES              # 512 output columns per core

TC = 512                         # token chunk for the QKV projection phase
QC = 256                         # query chunk in attention
N_HT = HID // 128                # 32 hidden tiles
N_QC = S // QC                   # 8 query chunks per batch
OPW = 512                        # oproj token group width
N_OPG = T // OPW                 # 8 oproj groups


def _emit(tc_ctx, xt, wqkvt, wot, ropes, out_t, qkt, ag_ins, ag_outs):
    nc = tc_ctx.nc
    n_ch = T // TC               # 8 qkv chunks
    n_kt = S // 128              # 16 k-tiles per batch

    with ExitStack() as es:
        const_pool = es.enter_context(tc_ctx.tile_pool(name="const", bufs=1))
        # All-ones stationary: one matmul both sums over the key partition
        # axis and broadcasts the sums across all 128 partitions.
        ones_mat = const_pool.tile([128, 128], F32R)
        # Diagonal causal masks, replicated for a 2-head pack (slots
        # [2*d_off + hp]): maskrep[k, 2*d+hp, q] = 1.0 iff q - k - 128*d >= 0.
        maskrep = const_pool.tile([128, 4, QC], BF16)
        nc.vector.memset(maskrep, 1.0)
        # memset on a float32r tile fails the ISA check; copy from the
        # all-ones bf16 tile instead.
        nc.vector.tensor_copy(ones_mat, maskrep[:, 0, 0:128])
        for d_off in range(2):
            for hp in range(2):
                nc.gpsimd.affine_select(
                    out=maskrep[:, 2 * d_off + hp, :],
                    in_=maskrep[:, 2 * d_off + hp, :],
                    compare_op=mybir.AluOpType.is_ge,
                    fill=0.0,
                    base=-128 * d_off,
                    pattern=[[1, QC]],
                    channel_multiplier=-1,
                )
        # Warm the ACT exp table before attention needs it.
        act_warm = const_pool.tile([128, 1], F32)
        nc.scalar.activation(act_warm, ones_mat[:, 0:1], EXP)

        # K/V/Q for attention, streamed from qkt per 512-token part so
        # attention pairs can start while the projection is still running.
        qpool = es.enter_context(tc_ctx.tile_pool(name="p2_q", bufs=4))
        kvpool = es.enter_context(tc_ctx.tile_pool(name="p2_kv", bufs=2))
        # Attention PSUM + SBUF pools span the whole kernel.
        ps_s = es.enter_context(
            tc_ctx.tile_pool(name="p2_ps_s", bufs=2, space="PSUM"))
        ps_o = es.enter_context(
            tc_ctx.tile_pool(name="p2_ps_o", bufs=1, space="PSUM"))
        ps_b = es.enter_context(
            tc_ctx.tile_pool(name="p2_ps_b", bufs=1, space="PSUM"))
        ptpool = es.enter_context(tc_ctx.tile_pool(name="p2_pt", bufs=4))
        cspool = es.enter_context(tc_ctx.tile_pool(name="p2_cs", bufs=2))
        mpool = es.enter_context(tc_ctx.tile_pool(name="p2_misc", bufs=2))

        kvq = {}
        qtiles = {}
        qkt_hh = qkt.rearrange("(hh p) t -> p hh t", p=128)
        pace = {"inst": None}

        def load_q(b, qc):
            q_t = qpool.tile([128, HL, QC], BF16, tag="q",
                             name=f"q_t{b}_{qc}")
            t0 = b * S + qc * QC
            nc.sync.dma_start(out=q_t, in_=qkt_hh[:, 0:HL, t0:t0 + QC])
            qtiles[(b, qc)] = q_t

        def alloc_kv(b):
            k_sb = kvpool.tile([128, S], BF16, tag="k", name=f"k_sb{b}")
            v_sb = kvpool.tile([128, n_kt, 128], BF16, tag="v",
                               name=f"v_sb{b}")
            kvq[b] = (k_sb, v_sb)

        def load_kq_part(b, p):
            """Load K/Q covering tokens [p*512, (p+1)*512) of batch b.
            V never round-trips through HBM (it is written straight into
            v_sb by phase A, already transposed)."""
            k_sb, _ = kvq[b]
            t0 = b * S + p * TC
            nc.sync.dma_start(
                out=k_sb[:, p * TC:(p + 1) * TC],
                in_=qkt[HL * 128:(HL + 1) * 128, t0:t0 + TC],
            )
            load_q(b, 2 * p)
            load_q(b, 2 * p + 1)

        def emit_attn(b, qc, filler=None):
            k_sb, v_sb = kvq[b]
            q_sb = qtiles.pop((b, qc))
            kt_max = 2 * qc + 2
            pso = [ps_o.tile([128, 2, QC], F32, tag=f"pso{hp}",
                             name=f"pso{hp}_{b}_{qc}")
                   for hp in range(2)]
            colsum = cspool.tile([128, 4, QC], F32R)
            pts = {}

            def emit_scores(kt, hp):
                # One N=512 matmul covers both heads of the pair.
                ps = ps_s.tile([128, 2, QC], F32)
                nc.tensor.matmul(
                    ps,
                    lhsT=k_sb[:, kt * 128:(kt + 1) * 128],
                    rhs=q_sb[:, hp * 2:hp * 2 + 2, :],
                    start=True,
                    stop=True,
                )
                pt = ptpool.tile([128, 2, QC], BF16)
                nc.scalar.activation(pt, ps, EXP)
                d_off = kt - 2 * qc
                if d_off >= 0:
                    nc.vector.tensor_mul(
                        pt, pt, maskrep[:, 2 * d_off:2 * d_off + 2, :]
                    )
                return pt

            def emit_pv(kt):
                for hp in range(2):
                    pt = pts.pop(kt)[hp] if hp == 1 else pts[kt][hp]
                    nc.tensor.matmul(
                        pso[hp],
                        lhsT=v_sb[:, kt, :],
                        rhs=pt,
                        start=(kt == 0),
                        stop=(kt == kt_max - 1),
                    )
                    if kt == 0:
                        nc.vector.tensor_copy(
                            colsum[:, hp * 2:hp * 2 + 2, :], pt
                        )
                    else:
                        nc.vector.tensor_add(
                            colsum[:, hp * 2:hp * 2 + 2, :],
                            colsum[:, hp * 2:hp * 2 + 2, :],
                            pt,
                        )

            for kt in range(kt_max):
                # Emit PV(kt-1) between the two score pairs of kt so the
                # in-order PE stream rides out the EXP latency with two
                # score-PSUM slots.
                cur = [emit_scores(kt, 0)]
                if kt >= 1:
                    emit_pv(kt - 1)
                cur.append(emit_scores(kt, 1))
                pts[kt] = cur
                if filler is not None and kt % 4 == 3:
                    # PE filler (o-proj blocks) while the serial EXP chain of
                    # the final pair catches up.
                    filler()
            emit_pv(kt_max - 1)

            # Partition-reduce + broadcast the denominators, then normalize
            # and ship to the AllGather input.
            sums_bc = ps_b.tile([128, 4, QC], F32)
            for hp in range(2):
                nc.tensor.matmul(
                    sums_bc[:, hp * 2:hp * 2 + 2, :],
                    lhsT=ones_mat,
                    rhs=colsum[:, hp * 2:hp * 2 + 2, :],
                    start=True,
                    stop=True,
                )
            recip = mpool.tile([128, 4, QC], F32, tag="recip")
            rscr = mpool.tile([128, 4, QC], F32, tag="rscr")
            nc.vector.reciprocal_approx_accurate(recip, sums_bc, rscr)
            ck = b * N_QC + qc
            ag_in_v = ag_ins[ck].rearrange("(hh p) t -> p hh t", p=128)
            for hp in range(2):
                attn_t = mpool.tile([128, 2, QC], BF16, tag=f"attn{hp}",
                                    name=f"attn{hp}_{b}_{qc}")
                nc.vector.tensor_mul(
                    attn_t, pso[hp], recip[:, hp * 2:hp * 2 + 2, :]
                )
                # The strided 512B-line writes are descriptor-bound; split
                # them across the sync and scalar queues so the collective's
                # inputs land in half the time.
                eng = nc.sync if hp == 0 else nc.scalar
                wr = eng.dma_start(
                    out=ag_in_v[:, hp * 2:hp * 2 + 2, :],
                    in_=attn_t,
                )
                pace["inst"] = wr.ins
            nc.gpsimd.collective_compute(
                "AllGather",
                mybir.AluOpType.bypass,
                replica_groups=[list(range(N_CORES))],
                ins=[ag_ins[ck][:]],
                outs=[ag_outs[ck][:]],
            )

        def emit_attn_pair(pair):
            b, p = divmod(pair, 4)
            emit_attn(b, 2 * p)
            emit_attn(b, 2 * p + 1)

        # ------ Phase A: QKV projection + RoPE, attention interleaved -------
        with tc_ctx.tile_pool(name="p1_w", bufs=1) as wpool, \
             tc_ctx.tile_pool(name="p1_x", bufs=2) as xpool, \
             tc_ctx.tile_pool(name="p1_rope", bufs=2) as rpool, \
             tc_ctx.tile_pool(name="p1_ps", bufs=2, space="PSUM") as pspool, \
             tc_ctx.tile_pool(name="p1_out", bufs=2) as opool, \
             tc_ctx.tile_pool(name="p1_sh", bufs=2) as shpool:
            wq_sb = wpool.tile([128, HL + 2, N_HT, 128], BF16)
            for ot in range(HL + 2):
                # Weights on the scalar-engine queue so the first X chunk
                # (sync queue) lands in parallel. Split ot=0 finely so the
                # very first matmuls start as soon as a slice arrives.
                if ot == 0:
                    for hq in range(4):
                        nc.scalar.dma_start(
                            out=wq_sb[:, 0, hq * 8:(hq + 1) * 8],
                            in_=wqkvt.ap()[:, 0, hq * 8:(hq + 1) * 8],
                        )
                else:
                    nc.scalar.dma_start(out=wq_sb[:, ot],
                                        in_=wqkvt.ap()[:, ot])
            for ch in range(n_ch):
                b, p = divmod(ch, S // TC)
                if p == 0:
                    alloc_kv(b)
                x_sb = xpool.tile([128, N_HT, TC], BF16)
                nsub = 4 if ch == 0 else 2
                for hq in range(nsub):
                    w = N_HT // nsub
                    nc.sync.dma_start(
                        out=x_sb[:, hq * w:(hq + 1) * w, :],
                        in_=xt.ap()[:, ch, hq * w:(hq + 1) * w, :],
                    )
                rope_sb = rpool.tile([128, 4, TC], BF16)
                nc.sync.dma_start(out=rope_sb, in_=ropes.ap()[:, ch])
                for ot in range(HL + 2):
                    if ot == HL + 1:
                        # V head, computed TRANSPOSED ([token, d]) by making
                        # the x slice the stationary operand: no HBM round
                        # trip and, critically, no DMA-transpose instructions
                        # (the tile framework serializes those against
                        # collectives, which stalls the whole pipeline).
                        psv = pspool.tile([128, 4, 128], F32, tag="ps")
                        for sub in range(4):
                            for h in range(N_HT):
                                # start=True clears has_written for the whole
                                # bank: only the first matmul sets it, later
                                # quarters overwrite via has_written==0.
                                nc.tensor.matmul(
                                    psv[:, sub, :],
                                    lhsT=x_sb[:, h,
                                              sub * 128:(sub + 1) * 128],
                                    rhs=wq_sb[:, ot, h, :],
                                    start=(sub == 0 and h == 0),
                                    stop=(sub == 3 and h == N_HT - 1),
                                )
                        v_sb = kvq[b][1]
                        nc.vector.tensor_copy(
                            v_sb[:, 4 * p:4 * p + 4, :], psv
                        )
                        continue
                    ps = pspool.tile([128, TC], F32, tag="ps")
                    for h in range(N_HT):
                        nc.tensor.matmul(
                            ps,
                            lhsT=wq_sb[:, ot, h, :],
                            rhs=x_sb[:, h, :],
                            start=(h == 0),
                            stop=(h == N_HT - 1),
                        )
                    # RoPE for Q (ot<HL, scaled tables) and K (ot==HL).
                    ci = 0 if ot < HL else 2
                    # sh = rotate_half(ps) * sin  (sign folded into sin)
                    sh = shpool.tile([128, TC], F32, tag="sh")
                    nc.vector.tensor_mul(
                        sh[0:64, :], ps[64:128, :], rope_sb[0:64, ci + 1, :]
                    )
                    nc.vector.tensor_mul(
                        sh[64:128, :], ps[0:64, :],
                        rope_sb[64:128, ci + 1, :]
                    )
                    tmp = shpool.tile([128, TC], F32, tag="tmp")
                    nc.vector.tensor_mul(tmp, ps, rope_sb[:, ci, :])
                    qk_out = opool.tile([128, TC], BF16, tag="qk")
                    nc.vector.tensor_add(qk_out, tmp, sh)
                    nc.sync.dma_start(
                        out=qkt[ot * 128:(ot + 1) * 128,
                                ch * TC:(ch + 1) * TC],
                        in_=qk_out,
                    )
                # Chunk ch of qkt is complete: stream the K/Q slices this
                # chunk produced, then run the attention pair that became
                # ready one chunk ago (its inputs had a full chunk to land).
                load_kq_part(b, p)
                if ch >= 1:
                    emit_attn_pair(ch - 1)

        # -------- Phase B: last attention pair + output projection ----------
        with tc_ctx.tile_pool(name="p4_w", bufs=1) as wopool, \
             tc_ctx.tile_pool(name="p4_a", bufs=2) as apool, \
             tc_ctx.tile_pool(name="p4_ps", bufs=2, space="PSUM") as ps4pool:
            wo_sb = wopool.tile([128, N_HT, JC], BF16)
            nc.sync.dma_start(out=wo_sb, in_=wot.ap())

            def load_ag(g):
                # No artificial pacing: by this point the AllGathers for
                # these chunks completed long ago (the CC stream runs during
                # phase A), so the real RAW dependency on ag_out suffices.
                ag_sb = apool.tile([128, N_HT, OPW], BF16, tag="ag",
                                   name=f"ag_sb{g}")
                for sub in range(2):
                    nc.scalar.dma_start(
                        out=ag_sb[:, :, sub * QC:(sub + 1) * QC],
                        in_=ag_outs[2 * g + sub].rearrange(
                            "(ht p) t -> p ht t", p=128),
                    )
                return ag_sb

            def emit_oproj_jt(g, ag_sb, jt):
                t0 = g * OPW
                ps4 = ps4pool.tile([128, OPW], F32)
                for h in range(N_HT):
                    nc.tensor.matmul(
                        ps4,
                        lhsT=wo_sb[:, h, jt * 128:(jt + 1) * 128],
                        rhs=ag_sb[:, h, :],
                        start=(h == 0),
                        stop=(h == N_HT - 1),
                    )
                res4 = apool.tile([128, OPW], F32, tag="res4")
                nc.vector.tensor_copy(res4, ps4)
                nc.sync.dma_start(
                    out=out_t[jt * 128:(jt + 1) * 128, t0:t0 + OPW],
                    in_=res4,
                )

            # Prefetch the first two gathered groups (their AllGathers are
            # long done); their jt-blocks double as PE filler inside the
            # final attention pair, whose serial EXP chain otherwise starves
            # the PE. Then stream the remaining groups, two loads in flight.
            ag_tiles = {0: load_ag(0), 1: load_ag(1)}
            consumed = dict.fromkeys(range(N_OPG), 0)
            filler_q = [(g, jt) for g in range(2) for jt in range(JC // 128)]

            def filler():
                if filler_q:
                    g, jt = filler_q.pop(0)
                    emit_oproj_jt(g, ag_tiles[g], jt)
                    consumed[g] = jt + 1

            b7, p7 = divmod(n_ch - 1, 4)
            emit_attn(b7, 2 * p7, filler=filler)
            emit_attn(b7, 2 * p7 + 1, filler=filler)
            for g in range(N_OPG):
                if g + 2 < N_OPG:
                    ag_tiles[g + 2] = load_ag(g + 2)
                ag_sb = ag_tiles.pop(g)
                for jt in range(consumed[g], JC // 128):
                    emit_oproj_jt(g, ag_sb, jt)


def _build_program():
    nc = bacc.Bacc("TRN2", target_bir_lowering=False, debug=False,
                   num_devices=N_CORES)
    xt = nc.declare_dram_parameter("xt", [128, T // TC, N_HT, TC], BF16,
                                   isOutput=False)
    wqkvt = nc.declare_dram_parameter("wqkvt", [128, HL + 2, N_HT, 128], BF16,
                                      isOutput=False)
    wot = nc.declare_dram_parameter("wot", [128, N_HT, JC], BF16,
                                    isOutput=False)
    ropes = nc.declare_dram_parameter("ropes", [128, T // TC, 4, TC], BF16,
                                      isOutput=False)
    out_t = nc.declare_dram_parameter("out_t", [JC, T], F32, isOutput=True)

    qkt = nc.dram_tensor("qkt", [QKV_ROWS, T], BF16).ap()
    ag_ins = [nc.dram_tensor(f"ag_in{k}", [HL * D, QC], BF16).ap()
              for k in range(T // QC)]
    ag_outs = [nc.dram_tensor(f"ag_out{k}", [N_HEADS * D, QC], BF16,
                              addr_space="Shared").ap()
               for k in range(T // QC)]

    with tile.TileContext(nc) as tc_ctx:
        _emit(tc_ctx, xt, wqkvt, wot, ropes, out_t, qkt, ag_ins, ag_outs)
    nc.finalize()
    return nc


def _host_inputs(hidden_states, w_qkv, w_o):
    """Shard + transpose inputs for the 8 cores; returns in_maps."""
    X = np.asarray(hidden_states, dtype=np.float32).reshape(T, HID)
    # [p, ch, ht, tc] tiled layout so every DMA line is contiguous.
    xt = np.ascontiguousarray(
        X.reshape(T // TC, TC, N_HT, 128).transpose(3, 0, 2, 1)
    ).astype(ml_dtypes.bfloat16)

    # RoPE tables in [d, t] layout with rotate-half sign folded into sin and
    # the attention scale folded into the Q tables.
    inv_freq = 1.0 / (ROPE_BASE ** (np.arange(0, D, 2, dtype=np.float32) / D))
    pos = np.arange(S, dtype=np.float32)
    freqs = np.outer(pos, inv_freq)                      # (S, D/2)
    emb = np.concatenate([freqs, freqs], axis=-1)        # (S, D)
    cos = np.cos(emb).T.astype(np.float32)               # (D, S)
    sin = np.sin(emb).T.astype(np.float32)
    sgn = np.concatenate([-np.ones(D // 2), np.ones(D // 2)]).astype(np.float32)
    sins = sgn[:, None] * sin
    cos_t = np.tile(cos, (1, B))                         # (D, T)
    sins_t = np.tile(sins, (1, B))
    scale = np.float32(D ** -0.5)
    ropes = np.stack([cos_t * scale, sins_t * scale, cos_t, sins_t], axis=0)
    ropes = np.ascontiguousarray(
        ropes.reshape(4, 128, T // TC, TC).transpose(1, 2, 0, 3)
    ).astype(ml_dtypes.bfloat16)

    w_qkv = np.asarray(w_qkv, dtype=np.float32)
    w_o = np.asarray(w_o, dtype=np.float32)
    q_sz = N_HEADS * D
    kv_sz = N_KV_HEADS * D
    in_maps = []
    for c in range(N_CORES):
        qr = w_qkv[c * HL * D:(c + 1) * HL * D]
        kr = w_qkv[q_sz + c * D:q_sz + (c + 1) * D]
        vr = w_qkv[q_sz + kv_sz + c * D:q_sz + kv_sz + (c + 1) * D]
        w_shard = np.concatenate([qr, kr, vr], axis=0)           # (768, HID)
        wqkvt_c = np.ascontiguousarray(
            w_shard.reshape(HL + 2, 128, N_HT, 128).transpose(3, 0, 2, 1)
        ).astype(ml_dtypes.bfloat16)
        wot_c = np.ascontiguousarray(
            w_o[c * JC:(c + 1) * JC, :].reshape(JC, N_HT, 128).transpose(2, 1, 0)
        ).astype(ml_dtypes.bfloat16)
        in_maps.append({
            "xt": xt, "wqkvt": wqkvt_c, "wot": wot_c, "ropes": ropes,
        })
    return in_maps


def _run(hidden_states, w_qkv, w_o, trace=False, tmpdir=None):
    in_maps = _host_inputs(hidden_states, w_qkv, w_o)
    nc = _build_program()
    res = run_bass_kernel_spmd(nc, in_maps, list(range(N_CORES)),
                               trace=trace, tmpdir=tmpdir)
    out_T = np.concatenate(
        [np.asarray(res.results[c]["out_t"]) for c in range(N_CORES)], axis=0
    )                                                     # (HID j, T)
    out = np.ascontiguousarray(out_T.T).reshape(B, S, HID).astype(np.float32)
    return out, res


def kernel(hidden_states, w_qkv, w_o):
    out, _ = _run(hidden_states, w_qkv, w_o, trace=False)
    return out

